# revision 11
# baseline (speedup 1.0000x reference)
"""Deformable Conv2d (3x3, stride 1, pad 1) on 8 Trainium2 NeuronCores.

Data-parallel over batch: core b handles sample b.

Wall-clock is dominated by the axon host<->device tunnel (~80-100MB/s
H2D, ~60MB/s D2H, ~40-70ms fixed cost per transfer/launch), so the
runner is organized to move the minimum bytes per call:
  - x ships bf16 (int8 was measured at rel-err 1.99e-2, too close to
    the 2e-2 bound), packed in ONE blob with the packed weights/biases.
  - the grid constant (input-independent) is device-resident, uploaded
    once at first call.
  - output buffers are donated and recycled across calls (never
    uploaded); the result returns int8 with a per-core f32 scale
    computed on device and embedded in the output tensor's last 4
    bytes (a separate tiny output would cost a ~60ms RPC round trip).

Per-core device pipeline (channel-major layout, C=128 on partitions):
  1. blob -> zero-padded x_pad [128, 100*100+pad] bf16
  2. 4-corner texture V [128, 10000, 4] bf16: V[:, j, m] = x_pad[j + {0,1,100,101}[m]]
  3. offset conv via 9 accumulating bf16 matmuls; stationary weights
     rebuilt on device so the 18 offset channels are replicated in all
     four 32-partition quadrants (enables stream_shuffle broadcast)
  4. DVE pipeline: p2 = off + grid + 2 (clamped), floor/frac split,
     flat corner index = 100*iy + ix (int16), frac tensor wY bf16
  5. per tap: wrapped idx layout for ap_gather (8 small DMAs)
  6. per (chunk, tap): stream_shuffle-broadcast bilinear weights, ap_gather
     4 corners, weighted-sum on DVE (bf16 S), accumulate taps into PSUM via
     bf16 matmul with conv_w, add bias, stash f32 chunk in DRAM scratch
     while accumulating the output absmax
  7. quantize: qscale = absmax/126; out int8 = scratch * (1/qscale)
"""
import numpy as np
import ml_dtypes
from contextlib import ExitStack

import concourse.bass as bass
import concourse.bacc as bacc
import concourse.tile as tile
import concourse.mybir as mybir

F32 = mybir.dt.float32
BF16 = mybir.dt.bfloat16
I16 = mybir.dt.int16
I32 = mybir.dt.int32
I8 = mybir.dt.int8

B, C, H, W, O = 8, 128, 96, 96, 128
K = 3
K2 = 9
N = H * W              # 9216 positions
PW = 100               # padded width/height
NPOS = PW * PW         # 10000
XPAD = NPOS + 104      # over-alloc so V-build shifted reads stay in bounds
NCHUNK = 6
CH = N // NCHUNK       # 1536 positions per chunk
ROWT = 24              # offset-conv tiles (4 rows x 96 cols = 384)
CLAMP_HI = 96.996 + 2.0  # clamp on p2 = py + 2
NGROUPS = 1            # core-groups (staggered overlap measured slower)

# weight-tail layout (bf16 columns appended to the packed-x planes)
WT_WW = 0                  # 1152 cols: ww[c, k*128+o] = conv_w[o, c, k]
WT_LOW = WT_WW + K2 * O    # 162 cols: low[c, k*18+ch] = offset_w[ch, c, k]
WT_OB = WT_LOW + K2 * 18   # 1 col (quadrant-replicated offset bias)
WT_CB = WT_OB + 1          # 1 col (conv bias)
WT_XS = WT_CB + 1          # 1 col (x dequant scale, replicated on all lanes)
WT = WT_XS + 1             # 1318 bf16 cols
NH = N // 2                # 4608 12-bit pairs per partition
NBX = 3 * NH               # 13824 packed x bytes per partition
NB8 = NBX + 2 * WT         # uint8 blob columns
NOUT = N + 4               # int8 out columns (last 4 bytes: f32 qscale on row 0)
UCH = 4                    # unpack chunks
UNH = NH // UCH            # 1536 pairs per unpack chunk

AG = mybir.AluOpType

_CACHE = {}


def _build():
    nc = bacc.Bacc("TRN2", target_bir_lowering=False, debug=False, num_devices=8)
    blob_in = nc.dram_tensor("blob", [C, NB8], mybir.dt.uint8, kind="ExternalInput").ap()
    grid_in = nc.dram_tensor("grid", [128, N], F32, kind="ExternalInput").ap()
    out_d = nc.dram_tensor("out", [128, NOUT], I8, kind="ExternalOutput").ap()

    PCH = 384  # pipeline chunk
    outscr = nc.dram_tensor("out_scratch", [128, N], F32, kind="Internal")
    bscr = nc.dram_tensor("bcast_scratch", [1, 1], F32, kind="Internal")

    with tile.TileContext(nc) as tc, ExitStack() as ctx:
        persist = ctx.enter_context(tc.tile_pool(name="persist", bufs=1))
        V = persist.tile([128, 4 * NPOS], BF16)
        V3 = V[:].rearrange("p (n d) -> p n d", d=4)
        wY = persist.tile([128, N], BF16)
        flat16 = persist.tile([128, N], I16)
        idxw = persist.tile([128, K2 * 576], I16)
        wtail = persist.tile([128, WT], BF16)
        nc.sync.dma_start(wtail[:], blob_in[:, NBX:NB8].bitcast(BF16))
        cbp = persist.tile([128, 1], F32)
        nc.scalar.copy(cbp[:], wtail[:, WT_CB:WT_CB + 1])
        amax = persist.tile([128, NCHUNK], F32)

        with tc.tile_pool(name="pool1", bufs=1) as pool1:
            # --- load + unpack 12-bit x into padded buffer ---
            # host packs pairs (a,b) as planes B0=a&255, B1=(a>>8)|((b&15)<<4),
            # B2=b>>4 with a,b = rint(x/s)+2047; even/odd image columns.
            x_pad = pool1.tile([128, XPAD], BF16)
            nc.vector.memset(x_pad[:], 0.0)
            xsc = pool1.tile([128, 1], F32)
            nc.scalar.copy(xsc[:], wtail[:, WT_XS:WT_XS + 1])
            X8 = pool1.tile([128, NBX], mybir.dt.uint8)
            nc.sync.dma_start(X8[:], blob_in[:, 0:NBX])
            HC = H // UCH  # image rows per unpack chunk
            with tc.tile_pool(name="unpack", bufs=1) as upool:
                for uc in range(UCH):
                    usl = slice(uc * UNH, (uc + 1) * UNH)
                    U0 = upool.tile([128, UNH], I16, tag="U0")
                    nc.vector.tensor_copy(U0[:], X8[:, usl])
                    U1 = upool.tile([128, UNH], I16, tag="U1")
                    nc.vector.tensor_copy(U1[:], X8[:, NH + uc * UNH:
                                                    NH + (uc + 1) * UNH])
                    U2 = upool.tile([128, UNH], I16, tag="U2")
                    nc.vector.tensor_copy(U2[:], X8[:, 2 * NH + uc * UNH:
                                                    2 * NH + (uc + 1) * UNH])
                    lo8 = upool.tile([128, UNH], I16, tag="lo8")
                    nc.vector.tensor_scalar(lo8[:], U1[:], 15, 8,
                                            op0=AG.bitwise_and,
                                            op1=AG.logical_shift_left)
                    hi = upool.tile([128, UNH], I16, tag="hi")
                    nc.vector.tensor_scalar(hi[:], U1[:], 4, 0,
                                            op0=AG.logical_shift_right,
                                            op1=AG.logical_shift_left)
                    u24 = upool.tile([128, UNH], I16, tag="u24")
                    nc.vector.tensor_scalar(u24[:], U2[:], 4, 0,
                                            op0=AG.logical_shift_left,
                                            op1=AG.logical_shift_left)
                    a = upool.tile([128, UNH], I16, tag="a")
                    nc.vector.tensor_tensor(a[:], U0[:], lo8[:], op=AG.add)
                    b = upool.tile([128, UNH], I16, tag="b")
                    nc.vector.tensor_tensor(b[:], hi[:], u24[:], op=AG.add)
                    am = upool.tile([128, UNH], I16, tag="am")
                    nc.vector.tensor_scalar(am[:], a[:], 2047, 0,
                                            op0=AG.subtract, op1=AG.add)
                    bm = upool.tile([128, UNH], I16, tag="bm")
                    nc.vector.tensor_scalar(bm[:], b[:], 2047, 0,
                                            op0=AG.subtract, op1=AG.add)
                    pofs = x_pad.offset + (uc * HC + 2) * PW + 2
                    nc.vector.tensor_scalar(
                        bass.AP(x_pad.tensor, pofs,
                                [[XPAD, 128], [PW, HC], [2, W // 2]]),
                        am[:].rearrange("c (h w) -> c h w", h=HC),
                        xsc[:], 0.0, op0=AG.mult, op1=AG.add)
                    nc.vector.tensor_scalar(
                        bass.AP(x_pad.tensor, pofs + 1,
                                [[XPAD, 128], [PW, HC], [2, W // 2]]),
                        bm[:].rearrange("c (h w) -> c h w", h=HC),
                        xsc[:], 0.0, op0=AG.mult, op1=AG.add)
            # offset-conv stationary: rebuild quadrant-replicated layout
            low = pool1.tile([128, K2 * 128], BF16)
            nc.vector.memset(low[:], 0.0)
            for k in range(K2):
                for q in range(4):
                    nc.scalar.copy(
                        low[:, k * 128 + 32 * q: k * 128 + 32 * q + 18],
                        wtail[:, WT_LOW + k * 18: WT_LOW + (k + 1) * 18])
            obp = pool1.tile([128, 1], F32)
            nc.scalar.copy(obp[:], wtail[:, WT_OB:WT_OB + 1])

            # --- 4-corner texture V (bf16) ---
            for m, dlt in enumerate((0, 1, PW, PW + 1)):
                nc.scalar.copy(
                    V3[:, :, m],
                    bass.AP(x_pad.tensor, x_pad.offset + dlt,
                            [[XPAD, 128], [1, NPOS]]))

            # --- offset conv (quadrant-replicated channels) ---
            offs = pool1.tile([128, N], BF16)
            with tc.tile_pool(name="ps_off", bufs=2, space="PSUM") as ps_off:
                for t in range(ROWT):
                    ps = ps_off.tile([128, 384], F32)
                    for a in range(K):
                        for b in range(K):
                            kk = a * K + b
                            rhs = bass.AP(
                                x_pad.tensor,
                                x_pad.offset + (4 * t + a) * PW + b + PW + 1,
                                [[XPAD, 128], [PW, 4], [1, W]])
                            nc.tensor.matmul(
                                ps[:], low[:, kk * 128:(kk + 1) * 128], rhs,
                                start=(kk == 0), stop=(kk == 8))
                    nc.vector.tensor_scalar(
                        offs[:, t * 384:(t + 1) * 384], ps[:], obp[:], 0.0,
                        op0=AG.add, op1=AG.add)

            # --- index/weight pipeline ---
            mask_xe = [min(i + 1, 31) if i % 2 == 0 else i for i in range(32)]
            with tc.tile_pool(name="pipe", bufs=1) as pipe:
                for cchunk in range(N // PCH):
                    sl = slice(cchunk * PCH, (cchunk + 1) * PCH)
                    g = pipe.tile([128, PCH], F32, tag="g")
                    nc.sync.dma_start(g[:], grid_in[:, sl])
                    t0 = pipe.tile([128, PCH], F32, tag="t0")
                    nc.vector.tensor_add(t0[:], offs[:, sl], g[:])
                    t1 = pipe.tile([128, PCH], F32, tag="t1")
                    nc.vector.tensor_scalar(t1[:], t0[:], CLAMP_HI, 0.0,
                                            op0=AG.min, op1=AG.max)
                    i0 = pipe.tile([128, PCH], I32, tag="i0")
                    nc.vector.tensor_copy(i0[:], t1[:])
                    f0 = pipe.tile([128, PCH], F32, tag="f0")
                    nc.vector.tensor_copy(f0[:], i0[:])
                    gt = pipe.tile([128, PCH], F32, tag="gt")
                    nc.vector.tensor_tensor(gt[:], f0[:], t1[:], op=AG.is_gt)
                    fl = pipe.tile([128, PCH], F32, tag="fl")
                    nc.vector.tensor_sub(fl[:], f0[:], gt[:])
                    nc.vector.tensor_sub(wY[:, sl], t1[:], fl[:])
                    fx = pipe.tile([128, PCH], F32, tag="fx")
                    nc.vector.stream_shuffle(fx[:], fl[:], mask_xe)
                    ff = pipe.tile([128, PCH], F32, tag="ff")
                    nc.vector.scalar_tensor_tensor(
                        ff[:], fl[:], 100.0, fx[:], op0=AG.mult, op1=AG.add)
                    nc.vector.tensor_copy(flat16[:, sl], ff[:])

        # --- wrapped idx layout: idxw[16g+r, k*576+f] = flat16[2k, 16f+r] ---
        # bounce through DRAM scratch (free-form APs) to cross partitions
        dscr = nc.dram_tensor("idx_scratch", [K2, N], I16, kind="Internal")
        for k in range(K2):
            nc.sync.dma_start(
                bass.AP(dscr, k * N, [[N, 1], [1, N]]),
                flat16[2 * k:2 * k + 1, :])
        for k in range(K2):
            src = bass.AP(dscr, k * N, [[1, 16], [16, 576]])
            for gq in range(8):
                nc.sync.dma_start(
                    idxw[16 * gq:16 * (gq + 1), k * 576:(k + 1) * 576], src)

        # --- main loop: chunks x taps ---
        with tc.tile_pool(name="gpool", bufs=2) as gpool, \
             tc.tile_pool(name="work", bufs=1) as work, \
             tc.tile_pool(name="outp", bufs=1) as outp, \
             tc.tile_pool(name="ps_main", bufs=2, space="PSUM") as ps_main:
            for cchunk in range(NCHUNK):
                sl = slice(cchunk * CH, (cchunk + 1) * CH)
                ps = ps_main.tile([128, CH], F32)
                for k in range(K2):
                    wyb = work.tile([128, CH], BF16, tag="wyb")
                    nc.vector.stream_shuffle(wyb[:], wY[:, sl], [2 * k] * 32)
                    wxb = work.tile([128, CH], BF16, tag="wxb")
                    nc.vector.stream_shuffle(wxb[:], wY[:, sl], [2 * k + 1] * 32)
                    G = gpool.tile([128, CH * 4], BF16, tag="G")
                    G3 = G[:].rearrange("p (n d) -> p n d", d=4)
                    nc.gpsimd.ap_gather(
                        G3, V3,
                        idxw[:, k * 576 + 96 * cchunk: k * 576 + 96 * (cchunk + 1)],
                        channels=128, num_elems=NPOS, d=4, num_idxs=CH)
                    uy = work.tile([128, CH], F32, tag="uy")
                    nc.vector.tensor_scalar(uy[:], wyb[:], -1.0, 1.0,
                                            op0=AG.mult, op1=AG.add)
                    ux = work.tile([128, CH], F32, tag="ux")
                    nc.vector.tensor_scalar(ux[:], wxb[:], -1.0, 1.0,
                                            op0=AG.mult, op1=AG.add)
                    S = work.tile([128, CH], BF16, tag="S")
                    for m, (wa, wb_) in enumerate(((uy, ux), (uy, wxb),
                                                   (wyb, ux), (wyb, wxb))):
                        p = work.tile([128, CH], F32, tag="p")
                        nc.vector.tensor_mul(p[:], wa[:], wb_[:])
                        if m == 0:
                            nc.vector.tensor_mul(S[:], p[:], G3[:, :, m])
                        else:
                            mm = work.tile([128, CH], F32, tag="mm")
                            nc.vector.tensor_mul(mm[:], p[:], G3[:, :, m])
                            nc.vector.tensor_add(S[:], S[:], mm[:])
                    for j in range(CH // 512):
                        nc.tensor.matmul(
                            ps[:, 512 * j:512 * (j + 1)],
                            wtail[:, k * 128:(k + 1) * 128],
                            S[:, 512 * j:512 * (j + 1)],
                            start=(k == 0), stop=(k == 8))
                ob = outp.tile([128, CH], F32, tag="ob")
                nc.vector.tensor_scalar(ob[:], ps[:], cbp[:], 0.0,
                                        op0=AG.add, op1=AG.add)
                nc.sync.dma_start(bass.AP(outscr, cchunk * CH,
                                          [[N, 128], [1, CH]]), ob[:])
                nc.vector.tensor_reduce(
                    amax[:, cchunk:cchunk + 1], ob[:], axis=mybir.AxisListType.X,
                    op=AG.max, apply_absolute_value=True)

            # --- output quantization scale (per-core global absmax) ---
            am1 = outp.tile([1, 1], F32, tag="am1")
            nc.gpsimd.tensor_reduce(am1[:], amax[:],
                                    axis=mybir.AxisListType.XYZWC, op=AG.max)
            qs = outp.tile([1, 1], F32, tag="qs")
            nc.vector.tensor_scalar(qs[:], am1[:], 1.0 / 126.0, 0.0,
                                    op0=AG.mult, op1=AG.add)
            nc.sync.dma_start(out_d[0:1, N:N + 4].bitcast(F32), qs[:])
            qr = outp.tile([1, 1], F32, tag="qr")
            nc.vector.reciprocal(qr[:], qs[:])
            nc.sync.dma_start(bass.AP(bscr, 0, [[1, 1], [1, 1]]), qr[:])
            qrb = outp.tile([128, 1], F32, tag="qrb")
            nc.sync.dma_start(qrb[:], bass.AP(bscr, 0, [[0, 128], [1, 1]]))

            # --- quantize scratch -> int8 out ---
            for cchunk in range(NCHUNK):
                sl = slice(cchunk * CH, (cchunk + 1) * CH)
                tq = outp.tile([128, CH], F32, tag="tq")
                nc.sync.dma_start(tq[:], bass.AP(outscr, cchunk * CH,
                                                 [[N, 128], [1, CH]]))
                oq = outp.tile([128, CH], I8, tag="oq")
                nc.vector.tensor_scalar(oq[:], tq[:], qrb[:], 0.0,
                                        op0=AG.mult, op1=AG.add)
                nc.sync.dma_start(out_d[:, sl], oq[:])
    nc.compile()
    return nc


def _make_grid():
    # grid const: lane 2k: y + 1 + ky + 2 ; lane 2k+1: x + 1 + kx + 2
    # p2 = off + (orig + 2): py = (y-1) + ky + off -> p2 = y + 1 + ky + off
    yy, xx = np.meshgrid(np.arange(H), np.arange(W), indexing="ij")
    grid = np.zeros((128, N), np.float32)
    for q in range(4):
        for k in range(K2):
            ky, kx = k // 3, k % 3
            grid[32 * q + 2 * k] = (yy.reshape(-1) + 1 + ky).astype(np.float32)
            grid[32 * q + 2 * k + 1] = (xx.reshape(-1) + 1 + kx).astype(np.float32)
    return grid


def _pack_x12(blob, xf, inv, lo, hi):
    """Pack samples [lo,hi) of xf as 12-bit planes into blob."""
    q = np.rint(xf[lo:hi] * inv[lo:hi, None, None]).astype(np.int16)
    q += 2047
    np.clip(q, 0, 4094, out=q)  # bf16 scale rounding can push past 12 bits
    qu = q.astype(np.uint16)
    qa, qb = qu[:, :, 0::2], qu[:, :, 1::2]
    blob[lo:hi, :, 0:NH] = (qa & 255).astype(np.uint8)
    blob[lo:hi, :, NH:2 * NH] = ((qa >> 8) | ((qb & 15) << 4)).astype(np.uint8)
    blob[lo:hi, :, 2 * NH:3 * NH] = (qb >> 4).astype(np.uint8)


def _pack_blob(x, offset_w, offset_b, conv_w, conv_b):
    """Host-side packing -> one uint8 blob [B*C, NB8] (12-bit x planes +
    bf16 weight tail bytes)."""
    import concurrent.futures
    if "blob" not in _CACHE:
        _CACHE["blob"] = np.zeros((B, C, NB8), np.uint8)
    blob = _CACHE["blob"]
    xf = np.asarray(x, np.float32).reshape(B, C, N)
    amax = np.abs(xf).max(axis=(1, 2))
    s16 = (np.maximum(amax, 1e-30) / 2040.0).astype(ml_dtypes.bfloat16)
    inv = 1.0 / s16.astype(np.float32)
    if "pool" not in _CACHE:
        _CACHE["pool"] = concurrent.futures.ThreadPoolExecutor(4)
    futs = [_CACHE["pool"].submit(_pack_x12, blob, xf, inv, i * 2, i * 2 + 2)
            for i in range(4)]

    wt = np.zeros((B, C, WT), ml_dtypes.bfloat16)
    wtc = np.zeros((C, WT), np.float32)
    wtc[:, WT_WW:WT_LOW] = np.asarray(conv_w, np.float32).reshape(O, C, K2) \
        .transpose(1, 2, 0).reshape(C, K2 * O)
    wtc[:, WT_LOW:WT_OB] = np.asarray(offset_w, np.float32) \
        .reshape(18, C, K2).transpose(1, 2, 0).reshape(C, K2 * 18)
    obcol = np.zeros(128, np.float32)
    for q in range(4):
        obcol[32 * q:32 * q + 18] = np.asarray(offset_b, np.float32)
    wtc[:, WT_OB] = obcol
    wtc[:, WT_CB] = np.asarray(conv_b, np.float32).reshape(128)
    wt[:] = wtc[None]
    wt[:, :, WT_XS] = s16[:, None]
    blob[:, :, NBX:] = wt.view(np.uint8)
    for f in futs:
        f.result()
    return blob.reshape(B * C, NB8)


def _make_runner(nc, n_cores):
    """Jitted PJRT runners on two 4-core groups (samples 0-3 / 4-7).

    The two groups run staggered: group B's blob upload starts as soon
    as group A's upload lands, so B's H2D overlaps A's exec + D2H fetch
    (the relay is partially duplex). Grid constants are device-resident;
    output buffers are donated and recycled across calls."""
    import threading
    import jax
    import jax.numpy as jnp
    from jax.sharding import Mesh, PartitionSpec, NamedSharding
    from jax.experimental.shard_map import shard_map
    from concourse.bass2jax import (
        _bass_exec_p, install_neuronx_cc_hook, partition_id_tensor)

    install_neuronx_cc_hook()
    partition_name = nc.partition_id_tensor.name if nc.partition_id_tensor else None
    in_names, out_names, out_avals = [], [], []
    for alloc in nc.m.functions[0].allocations:
        if not isinstance(alloc, mybir.MemoryLocationSet):
            continue
        name = alloc.memorylocations[0].name
        if alloc.kind == "ExternalInput":
            if name != partition_name and (nc.dbg_addr is None
                                           or name != nc.dbg_addr.name):
                in_names.append(name)
        elif alloc.kind == "ExternalOutput":
            out_names.append(name)
            shape = tuple(alloc.tensor_shape)
            dtype = mybir.dt.np(alloc.dtype)
            out_avals.append(jax.core.ShapedArray(shape, dtype))
    assert in_names == ["blob", "grid"], in_names
    assert out_names == ["out"], out_names
    n_params = len(in_names)
    all_in_names = list(in_names) + list(out_names)
    if nc.dbg_addr is not None:
        all_in_names.append(nc.dbg_addr.name)
    if partition_name is not None:
        all_in_names.append(partition_name)

    def _body(*args):
        operands = list(args)
        if nc.dbg_addr is not None:
            operands.append(jnp.zeros((1, 2), jnp.uint32))
        if partition_name is not None:
            operands.append(partition_id_tensor())
        outs = _bass_exec_p.bind(
            *operands,
            out_avals=tuple(out_avals),
            in_names=tuple(all_in_names),
            out_names=tuple(out_names),
            lowering_input_output_aliases=(),
            sim_require_finite=False,
            sim_require_nnan=False,
            nc=nc,
        )
        return tuple(outs)

    devices = jax.devices()[:n_cores]
    n_args = n_params + len(out_names)
    gsize = n_cores // NGROUPS
    grid_np = np.broadcast_to(_make_grid(), (gsize, 128, N)).reshape(gsize * 128, N)
    groups = []
    for gi in range(NGROUPS):
        mesh = Mesh(np.asarray(devices[gi * gsize:(gi + 1) * gsize]), ("core",))
        shard = NamedSharding(mesh, PartitionSpec("core"))
        sharded = jax.jit(
            shard_map(_body, mesh=mesh,
                      in_specs=(PartitionSpec("core"),) * n_args,
                      out_specs=(PartitionSpec("core"),) * len(out_names),
                      check_rep=False),
            donate_argnums=tuple(range(n_params, n_args)),
            keep_unused=True)
        grid_dev = jax.device_put(grid_np, shard)
        jax.block_until_ready(grid_dev)
        zeros = jax.jit(
            lambda a=out_avals[0]: jnp.zeros((gsize * a.shape[0], *a.shape[1:]),
                                             a.dtype),
            out_shardings=shard)
        groups.append({"shard": shard, "sharded": sharded, "grid": grid_dev,
                       "outbufs": (zeros(),)})

    def run(blob):
        results = [None] * NGROUPS
        errors = []
        events = [threading.Event() for _ in range(NGROUPS)]

        def work(gi):
            try:
                g = groups[gi]
                if gi > 0:
                    events[gi - 1].wait()
                bd = jax.device_put(blob[gi * gsize * C:(gi + 1) * gsize * C],
                                    g["shard"])
                outs = g["sharded"](bd, g["grid"], *g["outbufs"])
                jax.block_until_ready(bd)  # upload done -> next group may start
                events[gi].set()
                r = np.asarray(outs[0]).reshape(gsize, O, NOUT)
                g["outbufs"] = outs
                sc = np.ascontiguousarray(r[:, 0, N:N + 4]).view(np.float32)
                o = r[:, :, :N].astype(np.float32)
                o *= sc.reshape(gsize, 1, 1)
                results[gi] = o
            except BaseException as e:  # noqa: BLE001
                errors.append(e)
                events[gi].set()

        threads = [threading.Thread(target=work, args=(gi,))
                   for gi in range(NGROUPS)]
        for t in threads:
            t.start()
        for t in threads:
            t.join()
        if errors:
            raise errors[0]
        return np.concatenate(results, axis=0)
    return run


def kernel(x, offset_w, offset_b, conv_w, conv_b):
    if "nc" not in _CACHE:
        _CACHE["nc"] = _build()
    if "run" not in _CACHE:
        _CACHE["run"] = _make_runner(_CACHE["nc"], 8)
    blob = _pack_blob(x, offset_w, offset_b, conv_w, conv_b)
    out = _CACHE["run"](blob)
    return out.reshape(B, O, H, W)


if __name__ == "__main__":
    rng = np.random.default_rng(0)
    x = rng.standard_normal((B, C, H, W)).astype(np.float32)
    ow = (rng.standard_normal((18, C, K, K)) * 0.01).astype(np.float32)
    ob_ = (rng.standard_normal(18) * 0.01).astype(np.float32)
    cw = (rng.standard_normal((O, C, K, K)) / np.sqrt(C * 9)).astype(np.float32)
    cb_ = (rng.standard_normal(128) * 0.01).astype(np.float32)
    y = kernel(x, ow, ob_, cw, cb_)
    print("out", y.shape, y.dtype, float(np.abs(y).max()))


# revision 13
# speedup vs baseline: 1.4237x; 1.4237x over previous
"""Deformable Conv2d (3x3, stride 1, pad 1) on 8 Trainium2 NeuronCores.

Data-parallel over batch: core b handles sample b.

Wall-clock is dominated by the axon host<->device tunnel (~75-105MB/s
H2D, ~40-60MB/s D2H, ~40-70ms fixed cost per transfer/launch round
trip), so the runner is organized to move the minimum bytes per call:
  - x ships bf16 (int8 x was measured at rel-err 1.99e-2, too close to
    the 2e-2 bound; 12-bit packing saves 4.7MB but its host-side pack
    cost eats the wire savings), in ONE blob with the packed weights.
  - the grid constant (input-independent) is device-resident, uploaded
    once at first call.
  - output buffers are donated and recycled across calls (never
    uploaded); the result returns int8 with a per-core f32 scale
    computed on device and embedded in the output tensor's last 4
    bytes (a separate tiny output would cost a ~60ms RPC round trip).
  - the fetch is per-shard with dequantization overlapped in worker
    threads while later shards are still on the wire.

Per-core device pipeline (channel-major layout, C=128 on partitions):
  1. blob -> zero-padded x_pad [128, 100*100+pad] bf16
  2. 4-corner texture V [128, 10000, 4] bf16: V[:, j, m] = x_pad[j + {0,1,100,101}[m]]
  3. offset conv via 9 accumulating bf16 matmuls; stationary weights
     rebuilt on device so the 18 offset channels are replicated in all
     four 32-partition quadrants (enables stream_shuffle broadcast)
  4. DVE pipeline: p2 = off + grid + 2 (clamped), floor/frac split,
     flat corner index = 100*iy + ix (int16), frac tensor wY bf16
  5. per tap: wrapped idx layout for ap_gather (8 small DMAs)
  6. per (chunk, tap): stream_shuffle-broadcast bilinear weights, ap_gather
     4 corners, weighted-sum on DVE (bf16 S), accumulate taps into PSUM via
     bf16 matmul with conv_w, add bias, stash f32 chunk in DRAM scratch
     while accumulating the output absmax
  7. quantize: qscale = absmax/126; out int8 = scratch * (1/qscale)
"""
import numpy as np
import ml_dtypes
from contextlib import ExitStack

import concourse.bass as bass
import concourse.bacc as bacc
import concourse.tile as tile
import concourse.mybir as mybir

F32 = mybir.dt.float32
BF16 = mybir.dt.bfloat16
I16 = mybir.dt.int16
I32 = mybir.dt.int32
I8 = mybir.dt.int8

B, C, H, W, O = 8, 128, 96, 96, 128
K = 3
K2 = 9
N = H * W              # 9216 positions
PW = 100               # padded width/height
NPOS = PW * PW         # 10000
XPAD = NPOS + 104      # over-alloc so V-build shifted reads stay in bounds
NCHUNK = 6
CH = N // NCHUNK       # 1536 positions per chunk
ROWT = 24              # offset-conv tiles (4 rows x 96 cols = 384)
CLAMP_HI = 96.996 + 2.0  # clamp on p2 = py + 2

# weight-tail layout (bf16 columns appended to the bf16 x plane)
WT_WW = 0                  # 1152 cols: ww[c, k*128+o] = conv_w[o, c, k]
WT_LOW = WT_WW + K2 * O    # 162 cols: low[c, k*18+ch] = offset_w[ch, c, k]
WT_OB = WT_LOW + K2 * 18   # 1 col (quadrant-replicated offset bias)
WT_CB = WT_OB + 1          # 1 col (conv bias)
WT = WT_CB + 1             # 1316 bf16 cols
NCOL = N + WT              # bf16 blob columns (10532)
NOUT = N + 4               # int8 out columns (last 4 bytes: f32 qscale on row 0)

AG = mybir.AluOpType

_CACHE = {}


def _build():
    nc = bacc.Bacc("TRN2", target_bir_lowering=False, debug=False, num_devices=8)
    blob_in = nc.dram_tensor("blob", [C, NCOL], BF16, kind="ExternalInput").ap()
    grid_in = nc.dram_tensor("grid", [128, N], F32, kind="ExternalInput").ap()
    out_d = nc.dram_tensor("out", [128, NOUT], I8, kind="ExternalOutput").ap()

    PCH = 384  # pipeline chunk
    outscr = nc.dram_tensor("out_scratch", [128, N], F32, kind="Internal")
    bscr = nc.dram_tensor("bcast_scratch", [1, 1], F32, kind="Internal")

    with tile.TileContext(nc) as tc, ExitStack() as ctx:
        persist = ctx.enter_context(tc.tile_pool(name="persist", bufs=1))
        V = persist.tile([128, 4 * NPOS], BF16)
        V3 = V[:].rearrange("p (n d) -> p n d", d=4)
        wY = persist.tile([128, N], BF16)
        flat16 = persist.tile([128, N], I16)
        idxw = persist.tile([128, K2 * 576], I16)
        wtail = persist.tile([128, WT], BF16)
        nc.sync.dma_start(wtail[:], blob_in[:, N:NCOL])
        cbp = persist.tile([128, 1], F32)
        nc.scalar.copy(cbp[:], wtail[:, WT_CB:WT_CB + 1])
        amax = persist.tile([128, NCHUNK], F32)

        with tc.tile_pool(name="pool1", bufs=1) as pool1:
            # --- load x into padded buffer ---
            x_pad = pool1.tile([128, XPAD], BF16)
            nc.vector.memset(x_pad[:], 0.0)
            nc.sync.dma_start(
                bass.AP(x_pad.tensor, x_pad.offset + 2 * PW + 2,
                        [[XPAD, 128], [PW, H], [1, W]]),
                blob_in[:, 0:N].rearrange("c (h w) -> c h w", h=H))
            # offset-conv stationary: rebuild quadrant-replicated layout
            low = pool1.tile([128, K2 * 128], BF16)
            nc.vector.memset(low[:], 0.0)
            for k in range(K2):
                for q in range(4):
                    nc.scalar.copy(
                        low[:, k * 128 + 32 * q: k * 128 + 32 * q + 18],
                        wtail[:, WT_LOW + k * 18: WT_LOW + (k + 1) * 18])
            obp = pool1.tile([128, 1], F32)
            nc.scalar.copy(obp[:], wtail[:, WT_OB:WT_OB + 1])

            # --- 4-corner texture V (bf16) ---
            for m, dlt in enumerate((0, 1, PW, PW + 1)):
                nc.scalar.copy(
                    V3[:, :, m],
                    bass.AP(x_pad.tensor, x_pad.offset + dlt,
                            [[XPAD, 128], [1, NPOS]]))

            # --- offset conv (quadrant-replicated channels) ---
            offs = pool1.tile([128, N], BF16)
            with tc.tile_pool(name="ps_off", bufs=2, space="PSUM") as ps_off:
                for t in range(ROWT):
                    ps = ps_off.tile([128, 384], F32)
                    for a in range(K):
                        for b in range(K):
                            kk = a * K + b
                            rhs = bass.AP(
                                x_pad.tensor,
                                x_pad.offset + (4 * t + a) * PW + b + PW + 1,
                                [[XPAD, 128], [PW, 4], [1, W]])
                            nc.tensor.matmul(
                                ps[:], low[:, kk * 128:(kk + 1) * 128], rhs,
                                start=(kk == 0), stop=(kk == 8))
                    nc.vector.tensor_scalar(
                        offs[:, t * 384:(t + 1) * 384], ps[:], obp[:], 0.0,
                        op0=AG.add, op1=AG.add)

            # --- index/weight pipeline ---
            mask_xe = [min(i + 1, 31) if i % 2 == 0 else i for i in range(32)]
            with tc.tile_pool(name="pipe", bufs=1) as pipe:
                for cchunk in range(N // PCH):
                    sl = slice(cchunk * PCH, (cchunk + 1) * PCH)
                    g = pipe.tile([128, PCH], F32, tag="g")
                    nc.sync.dma_start(g[:], grid_in[:, sl])
                    t0 = pipe.tile([128, PCH], F32, tag="t0")
                    nc.vector.tensor_add(t0[:], offs[:, sl], g[:])
                    t1 = pipe.tile([128, PCH], F32, tag="t1")
                    nc.vector.tensor_scalar(t1[:], t0[:], CLAMP_HI, 0.0,
                                            op0=AG.min, op1=AG.max)
                    i0 = pipe.tile([128, PCH], I32, tag="i0")
                    nc.vector.tensor_copy(i0[:], t1[:])
                    f0 = pipe.tile([128, PCH], F32, tag="f0")
                    nc.vector.tensor_copy(f0[:], i0[:])
                    gt = pipe.tile([128, PCH], F32, tag="gt")
                    nc.vector.tensor_tensor(gt[:], f0[:], t1[:], op=AG.is_gt)
                    fl = pipe.tile([128, PCH], F32, tag="fl")
                    nc.vector.tensor_sub(fl[:], f0[:], gt[:])
                    nc.vector.tensor_sub(wY[:, sl], t1[:], fl[:])
                    fx = pipe.tile([128, PCH], F32, tag="fx")
                    nc.vector.stream_shuffle(fx[:], fl[:], mask_xe)
                    ff = pipe.tile([128, PCH], F32, tag="ff")
                    nc.vector.scalar_tensor_tensor(
                        ff[:], fl[:], 100.0, fx[:], op0=AG.mult, op1=AG.add)
                    nc.vector.tensor_copy(flat16[:, sl], ff[:])

        # --- wrapped idx layout: idxw[16g+r, k*576+f] = flat16[2k, 16f+r] ---
        # bounce through DRAM scratch (free-form APs) to cross partitions
        dscr = nc.dram_tensor("idx_scratch", [K2, N], I16, kind="Internal")
        for k in range(K2):
            nc.sync.dma_start(
                bass.AP(dscr, k * N, [[N, 1], [1, N]]),
                flat16[2 * k:2 * k + 1, :])
        for k in range(K2):
            src = bass.AP(dscr, k * N, [[1, 16], [16, 576]])
            for gq in range(8):
                nc.sync.dma_start(
                    idxw[16 * gq:16 * (gq + 1), k * 576:(k + 1) * 576], src)

        # --- main loop: chunks x taps ---
        with tc.tile_pool(name="gpool", bufs=2) as gpool, \
             tc.tile_pool(name="work", bufs=1) as work, \
             tc.tile_pool(name="outp", bufs=1) as outp, \
             tc.tile_pool(name="ps_main", bufs=2, space="PSUM") as ps_main:
            for cchunk in range(NCHUNK):
                sl = slice(cchunk * CH, (cchunk + 1) * CH)
                ps = ps_main.tile([128, CH], F32)
                for k in range(K2):
                    wyb = work.tile([128, CH], BF16, tag="wyb")
                    nc.vector.stream_shuffle(wyb[:], wY[:, sl], [2 * k] * 32)
                    wxb = work.tile([128, CH], BF16, tag="wxb")
                    nc.vector.stream_shuffle(wxb[:], wY[:, sl], [2 * k + 1] * 32)
                    G = gpool.tile([128, CH * 4], BF16, tag="G")
                    G3 = G[:].rearrange("p (n d) -> p n d", d=4)
                    nc.gpsimd.ap_gather(
                        G3, V3,
                        idxw[:, k * 576 + 96 * cchunk: k * 576 + 96 * (cchunk + 1)],
                        channels=128, num_elems=NPOS, d=4, num_idxs=CH)
                    uy = work.tile([128, CH], F32, tag="uy")
                    nc.vector.tensor_scalar(uy[:], wyb[:], -1.0, 1.0,
                                            op0=AG.mult, op1=AG.add)
                    ux = work.tile([128, CH], F32, tag="ux")
                    nc.vector.tensor_scalar(ux[:], wxb[:], -1.0, 1.0,
                                            op0=AG.mult, op1=AG.add)
                    S = work.tile([128, CH], BF16, tag="S")
                    for m, (wa, wb_) in enumerate(((uy, ux), (uy, wxb),
                                                   (wyb, ux), (wyb, wxb))):
                        p = work.tile([128, CH], F32, tag="p")
                        nc.vector.tensor_mul(p[:], wa[:], wb_[:])
                        if m == 0:
                            nc.vector.tensor_mul(S[:], p[:], G3[:, :, m])
                        else:
                            mm = work.tile([128, CH], F32, tag="mm")
                            nc.vector.tensor_mul(mm[:], p[:], G3[:, :, m])
                            nc.vector.tensor_add(S[:], S[:], mm[:])
                    for j in range(CH // 512):
                        nc.tensor.matmul(
                            ps[:, 512 * j:512 * (j + 1)],
                            wtail[:, k * 128:(k + 1) * 128],
                            S[:, 512 * j:512 * (j + 1)],
                            start=(k == 0), stop=(k == 8))
                ob = outp.tile([128, CH], F32, tag="ob")
                nc.vector.tensor_scalar(ob[:], ps[:], cbp[:], 0.0,
                                        op0=AG.add, op1=AG.add)
                nc.sync.dma_start(bass.AP(outscr, cchunk * CH,
                                          [[N, 128], [1, CH]]), ob[:])
                nc.vector.tensor_reduce(
                    amax[:, cchunk:cchunk + 1], ob[:], axis=mybir.AxisListType.X,
                    op=AG.max, apply_absolute_value=True)

            # --- output quantization scale (per-core global absmax) ---
            am1 = outp.tile([1, 1], F32, tag="am1")
            nc.gpsimd.tensor_reduce(am1[:], amax[:],
                                    axis=mybir.AxisListType.XYZWC, op=AG.max)
            qs = outp.tile([1, 1], F32, tag="qs")
            nc.vector.tensor_scalar(qs[:], am1[:], 1.0 / 126.0, 0.0,
                                    op0=AG.mult, op1=AG.add)
            nc.sync.dma_start(out_d[0:1, N:N + 4].bitcast(F32), qs[:])
            qr = outp.tile([1, 1], F32, tag="qr")
            nc.vector.reciprocal(qr[:], qs[:])
            nc.sync.dma_start(bass.AP(bscr, 0, [[1, 1], [1, 1]]), qr[:])
            qrb = outp.tile([128, 1], F32, tag="qrb")
            nc.sync.dma_start(qrb[:], bass.AP(bscr, 0, [[0, 128], [1, 1]]))

            # --- quantize scratch -> int8 out ---
            for cchunk in range(NCHUNK):
                sl = slice(cchunk * CH, (cchunk + 1) * CH)
                tq = outp.tile([128, CH], F32, tag="tq")
                nc.sync.dma_start(tq[:], bass.AP(outscr, cchunk * CH,
                                                 [[N, 128], [1, CH]]))
                oq = outp.tile([128, CH], I8, tag="oq")
                nc.vector.tensor_scalar(oq[:], tq[:], qrb[:], 0.0,
                                        op0=AG.mult, op1=AG.add)
                nc.sync.dma_start(out_d[:, sl], oq[:])
    nc.compile()
    return nc


def _make_grid():
    # grid const: lane 2k: y + 1 + ky + 2 ; lane 2k+1: x + 1 + kx + 2
    # p2 = off + (orig + 2): py = (y-1) + ky + off -> p2 = y + 1 + ky + off
    yy, xx = np.meshgrid(np.arange(H), np.arange(W), indexing="ij")
    grid = np.zeros((128, N), np.float32)
    for q in range(4):
        for k in range(K2):
            ky, kx = k // 3, k % 3
            grid[32 * q + 2 * k] = (yy.reshape(-1) + 1 + ky).astype(np.float32)
            grid[32 * q + 2 * k + 1] = (xx.reshape(-1) + 1 + kx).astype(np.float32)
    return grid


def _pack_blob(x, offset_w, offset_b, conv_w, conv_b):
    """Host-side packing -> one bf16 blob [B*C, NCOL]."""
    import concurrent.futures
    if "blob" not in _CACHE:
        _CACHE["blob"] = np.zeros((B, C, NCOL), ml_dtypes.bfloat16)
        _CACHE["pool"] = concurrent.futures.ThreadPoolExecutor(8)
    blob = _CACHE["blob"]
    xf = np.asarray(x, np.float32).reshape(B, C, N)
    futs = [_CACHE["pool"].submit(
        lambda i=i: blob.__setitem__((slice(i, i + 1), slice(None), slice(0, N)),
                                     xf[i:i + 1]))
        for i in range(B)]
    wtc = np.zeros((C, WT), np.float32)
    wtc[:, WT_WW:WT_LOW] = np.asarray(conv_w, np.float32).reshape(O, C, K2) \
        .transpose(1, 2, 0).reshape(C, K2 * O)
    wtc[:, WT_LOW:WT_OB] = np.asarray(offset_w, np.float32) \
        .reshape(18, C, K2).transpose(1, 2, 0).reshape(C, K2 * 18)
    obcol = np.zeros(128, np.float32)
    for q in range(4):
        obcol[32 * q:32 * q + 18] = np.asarray(offset_b, np.float32)
    wtc[:, WT_OB] = obcol
    wtc[:, WT_CB] = np.asarray(conv_b, np.float32).reshape(128)
    blob[:, :, N:] = wtc[None]
    for f in futs:
        f.result()
    return blob.reshape(B * C, NCOL)


def _make_runner(nc, n_cores):
    """Jitted PJRT runner: one bf16 blob shipped per call; grid constant
    device-resident; output buffers donated and recycled; per-shard fetch
    with dequantization overlapped in pool threads."""
    import concurrent.futures
    import jax
    import jax.numpy as jnp
    from jax.sharding import Mesh, PartitionSpec, NamedSharding
    from jax.experimental.shard_map import shard_map
    from concourse.bass2jax import (
        _bass_exec_p, install_neuronx_cc_hook, partition_id_tensor)

    install_neuronx_cc_hook()
    partition_name = nc.partition_id_tensor.name if nc.partition_id_tensor else None
    in_names, out_names, out_avals = [], [], []
    for alloc in nc.m.functions[0].allocations:
        if not isinstance(alloc, mybir.MemoryLocationSet):
            continue
        name = alloc.memorylocations[0].name
        if alloc.kind == "ExternalInput":
            if name != partition_name and (nc.dbg_addr is None
                                           or name != nc.dbg_addr.name):
                in_names.append(name)
        elif alloc.kind == "ExternalOutput":
            out_names.append(name)
            shape = tuple(alloc.tensor_shape)
            dtype = mybir.dt.np(alloc.dtype)
            out_avals.append(jax.core.ShapedArray(shape, dtype))
    assert in_names == ["blob", "grid"], in_names
    assert out_names == ["out"], out_names
    n_params = len(in_names)
    all_in_names = list(in_names) + list(out_names)
    if nc.dbg_addr is not None:
        all_in_names.append(nc.dbg_addr.name)
    if partition_name is not None:
        all_in_names.append(partition_name)

    def _body(*args):
        operands = list(args)
        if nc.dbg_addr is not None:
            operands.append(jnp.zeros((1, 2), jnp.uint32))
        if partition_name is not None:
            operands.append(partition_id_tensor())
        outs = _bass_exec_p.bind(
            *operands,
            out_avals=tuple(out_avals),
            in_names=tuple(all_in_names),
            out_names=tuple(out_names),
            lowering_input_output_aliases=(),
            sim_require_finite=False,
            sim_require_nnan=False,
            nc=nc,
        )
        return tuple(outs)

    devices = jax.devices()[:n_cores]
    mesh = Mesh(np.asarray(devices), ("core",))
    shard = NamedSharding(mesh, PartitionSpec("core"))
    n_args = n_params + len(out_names)
    sharded = jax.jit(
        shard_map(_body, mesh=mesh,
                  in_specs=(PartitionSpec("core"),) * n_args,
                  out_specs=(PartitionSpec("core"),) * len(out_names),
                  check_rep=False),
        donate_argnums=tuple(range(n_params, n_args)),
        keep_unused=True)

    grid_dev = jax.device_put(
        np.broadcast_to(_make_grid(), (n_cores, 128, N)).reshape(n_cores * 128, N),
        shard)
    jax.block_until_ready(grid_dev)
    # output buffer lives on device; each call donates the previous call's
    # (already fetched) buffer, so no H2D bytes are ever spent on it. The
    # kernel overwrites every output element, so stale contents are harmless.
    zeros = jax.jit(
        lambda a=out_avals[0]: jnp.zeros((n_cores * a.shape[0], *a.shape[1:]),
                                         a.dtype),
        out_shardings=shard)
    state = {"outbufs": (zeros(),)}
    pool = concurrent.futures.ThreadPoolExecutor(4)

    def _dequant(outbuf_np, i, r):
        sc = r[0, N:N + 4].copy().view(np.float32)[0]
        out = r[:, :N].astype(np.float32)
        out *= sc
        outbuf_np[i] = out

    def run(blob):
        blob_dev = jax.device_put(blob, shard)
        outs = sharded(blob_dev, grid_dev, *state["outbufs"])
        outbuf_np = np.empty((B, O, N), np.float32)
        shards = sorted(outs[0].addressable_shards,
                        key=lambda s: s.index[0].start or 0)
        for s in shards:
            s.data.copy_to_host_async()
        futs = []
        for i, s in enumerate(shards):
            r = np.asarray(s.data)  # blocks on this shard's D2H
            futs.append(pool.submit(_dequant, outbuf_np, i, r))
        state["outbufs"] = outs
        for f in futs:
            f.result()
        return outbuf_np
    return run


def kernel(x, offset_w, offset_b, conv_w, conv_b):
    if "nc" not in _CACHE:
        _CACHE["nc"] = _build()
    if "run" not in _CACHE:
        _CACHE["run"] = _make_runner(_CACHE["nc"], 8)
    blob = _pack_blob(x, offset_w, offset_b, conv_w, conv_b)
    out = _CACHE["run"](blob)
    return out.reshape(B, O, H, W)


if __name__ == "__main__":
    rng = np.random.default_rng(0)
    x = rng.standard_normal((B, C, H, W)).astype(np.float32)
    ow = (rng.standard_normal((18, C, K, K)) * 0.01).astype(np.float32)
    ob_ = (rng.standard_normal(18) * 0.01).astype(np.float32)
    cw = (rng.standard_normal((O, C, K, K)) / np.sqrt(C * 9)).astype(np.float32)
    cb_ = (rng.standard_normal(128) * 0.01).astype(np.float32)
    y = kernel(x, ow, ob_, cw, cb_)
    print("out", y.shape, y.dtype, float(np.abs(y).max()))
